# revision 21
# baseline (speedup 1.0000x reference)
"""Trainium2 Bass kernel for nn_AutoCorr2D.

Computation (per sample):
  f   = conv3x3(x, w_ext, pad=1) + b_ext            # [CC=128, 64, 64]
  corr[c,i,j,k] = f[c,i,j] * fpad[c, i+u-2, j+v-2]  # 5x5 window products
  out[o,i,j]    = sum_{c,k} w_reg[o,c,k] * corr[c,i,j,k] + b_reg[o]

Sharding: data-parallel over batch B=8 across 8 NeuronCores (one sample per
core); conv weights replicated.

Per-core implementation (all matmul operands bf16, PSUM accumulation f32;
end-to-end rel err ~5e-3 vs the f32 reference, tolerance 2e-2):
  stage 1: implicit GEMM over (cin_tile, 3x3 tap): 18 accumulating bf16
           matmuls per 512-pixel chunk reading shifted views of a
           zero-padded x buffer (DMA'd as bf16, pad-scattered by VectorE
           at 4x rate); bias folded into two PSUM->SBUF ScalarE copies
           that write the feature map twice: fpad (col offset 2) and fodd
           (col offset 1) so every stage-2 product has 4B-aligned operands
           (DVE 2x_1P mode needs 16-bit dtype + 4B alignment; taps with
           odd column shift read their shifted operand from fodd).
  stage 2: product symmetry: P_{a,b}[y,x] = f[y,x]*f[y+a,x+b] serves both
           tap (a,b) and tap (-a,-b) via shifted reads, so only 13 of 25
           product maps are computed per 2-chunk row group (ScalarE Square
           for (0,0) - emitted inside the stage-1 loop to dodge FIFO
           head-of-line blocking - VectorE tensor_mul at 2x for the rest).
           The regressor GEMM has M=COUT=64 - half the PE array - so the
           two chunks of each group run CONCURRENTLY via 2x column tiling:
           chunk A accumulates 25 taps into PSUM partitions 0-63
           (tile_position (0,0)), chunk B into partitions 64-127
           (tile_position (0,64)) of the same bank; one 128-partition
           bias-copy then two DMAs write both chunks out.
  The PE is pre-warmed with dummy matmuls so the HAM clock gate releases
  before real work.
"""

import numpy as np
import ml_dtypes

from concourse import bacc, mybir, tile
from concourse.bass_utils import run_bass_kernel_spmd

B, CIN, H, W = 8, 256, 64, 64
CC, COUT = 128, 64
HW = H * W
NCORES = 8

NCHUNK = 8           # pixel chunks per image
CROWS = H // NCHUNK  # rows per chunk (8) -> N = 512 pixels
NPX = CROWS * W      # 512
NGRP = 4             # product-map groups (2 chunks each)
GROWS = 2 * CROWS    # 16

XP = W + 4           # xpad cols: data at 2..65, zeros at 0,1,66,67 (4B align)
XR = H + 2           # xpad rows (pad=1)
FP = W + 4           # fpad cols (pad=2)
FR = H + 4           # fpad rows
FTAIL = 72           # guard tail so shifted product reads stay in-bounds

# The 13 "upper half" taps; (a,b) also serves tap (-a,-b) via a shifted read.
SYM = [(0, 0), (0, 1), (0, 2),
       (1, -2), (1, -1), (1, 0), (1, 1), (1, 2),
       (2, -2), (2, -1), (2, 0), (2, 1), (2, 2)]

F32 = mybir.dt.float32
BF16 = mybir.dt.bfloat16
U32 = mybir.dt.uint32
AF = mybir.ActivationFunctionType
BFNP = ml_dtypes.bfloat16


def build_body(nc, tc, x, wext, wreg, biases, out):
    with (
        tc.tile_pool(name="const", bufs=1) as constp,
        tc.tile_pool(name="xpadp", bufs=1) as xpadp,
        tc.tile_pool(name="fpadp", bufs=1) as fpadp,
        tc.tile_pool(name="prodp", bufs=3) as prodp,
        tc.tile_pool(name="outp", bufs=2) as outp,
        tc.tile_pool(name="ps1", bufs=3, space="PSUM") as ps1,
        tc.tile_pool(name="ps2a", bufs=2, space="PSUM") as ps2a,
        tc.tile_pool(name="ps2b", bufs=2, space="PSUM") as ps2b,
        tc.tile_pool(name="warmp", bufs=1, space="PSUM") as warmp,
    ):
        # PE warm-up: dummy matmuls on a zeroed bf16 scratch start immediately
        # and release the HAM clock gate (~3.4us of activity) before real
        # matmuls begin.
        wsc = constp.tile([128, NPX], BF16, name="wsc")
        nc.vector.memset(wsc.bitcast(U32), 0)
        wpsum = warmp.tile([128, NPX], F32, name="wpsum")
        NWARM = 6   # ends right when band 0 and wext block 0 have landed
        for i in range(NWARM):
            nc.tensor.matmul(wpsum, wsc[:, :128], wsc,
                             start=(i == 0), stop=(i == NWARM - 1))

        # ---- x pad buffers: the host pre-bakes x into the padded
        # [XR, XP] layout (zeros included), so bands DMA straight into
        # xpad slices - no staging, no DVE scatter, no border memsets.
        xpads = []
        xflat = []
        for t in range(2):
            xp = xpadp.tile([128, XR * XP], BF16, name=f"xpad{t}",
                            tag=f"xpad{t}")
            xflat.append(xp)
            xpads.append(xp.rearrange("p (r c) -> p r c", c=XP))

        # DMA queue plan (2 HWDGE queues: Sync + Scalar):
        #   Sync:   band0 (both halves) then bands 1-7, then stage 2's
        #           A-half output DMAs
        #   Scalar: wext in three 6-block pieces (2KB partition lines run
        #           ~1.7x faster than sub-1KB ones, and three completion
        #           semaphores beat eighteen), biases, wreg, stage 2's
        #           B-half output DMAs
        wext_sb = constp.tile([128, 18 * 128], BF16, name="wext_sb")
        wreg_sb = constp.tile([128, 25 * 64], BF16, name="wreg_sb")
        bias_sb = constp.tile([128, 2], F32, name="bias_sb")

        def dma_wext(q, lo, hi):
            q.dma_start(out=wext_sb[:, lo * 128:hi * 128],
                        in_=wext[:, lo * 128:hi * 128])

        # band row ranges in xpad coords: band 0 carries rows 0-10 (incl.
        # top pad), bands 1-6 the next 8 rows each, band 7 rows 59-65
        # (incl. bottom pad); non-overlapping, so later bands prefetch
        # with no WAR hazard against earlier chunks' reads.
        BANDS = [(0, 11)] + [(8 * i + 3, 8) for i in range(1, 7)] + [(59, 7)]

        def dma_band(band, t):
            r0, nr = BANDS[band]
            nc.sync.dma_start(
                out=xflat[t][:, r0 * XP:(r0 + nr) * XP],
                in_=x[t * 128:(t + 1) * 128, r0 * XP:(r0 + nr) * XP])

        dma_wext(nc.scalar, 0, 6)
        dma_band(0, 0)
        dma_band(0, 1)
        dma_wext(nc.scalar, 6, 12)
        dma_wext(nc.scalar, 12, 18)
        nc.scalar.dma_start(out=bias_sb, in_=biases)
        nc.scalar.dma_start(out=wreg_sb, in_=wreg)
        for band in range(1, NCHUNK):
            dma_band(band, 0)
            dma_band(band, 1)

        if True:
            # ---- padded features (pad=2) + guard tail; fodd is the same
            # map shifted one column left (fodd[r,x] = fpad[r,x+1]) so
            # odd-column-shift product operands stay 4B-aligned ----
            fpad = fpadp.tile([128, FR * FP + FTAIL], BF16, name="fpad")
            fodd = fpadp.tile([128, FR * FP + FTAIL], BF16, name="fodd")
            fr = fpad[:, :FR * FP].rearrange("p (r c) -> p r c", c=FP)
            fo = fodd[:, :FR * FP].rearrange("p (r c) -> p r c", c=FP)

            # ---- stage 1: f = conv3x3(x) + b_ext ----
            # fpad/fodd border memsets go first in VectorE's FIFO (cheap,
            # products need them later); the center-tap Square for group g
            # is emitted right after chunk 2g+1's bias-copies so ScalarE
            # reaches it long before the PE needs it (no head-of-line
            # blocking behind later chunks' copies).
            squares = [None] * NGRP
            for i in range(NCHUNK):
                if i == 0:
                    fpi = fpad.bitcast(U32)
                    foi = fodd.bitcast(U32)
                    fri = fpi[:, :FR * FP // 2].rearrange(
                        "p (r c) -> p r c", c=FP // 2)
                    nc.vector.memset(fpi[:, 0:FP], 0)
                    nc.vector.memset(fpi[:, (FR - 2) * FP // 2:
                                          (FR * FP + FTAIL) // 2], 0)
                    nc.vector.memset(fri[:, 2:FR - 2, 0], 0)
                    nc.vector.memset(fri[:, 2:FR - 2, FP // 2 - 1], 0)
                    nc.vector.memset(foi[:, 0:FP], 0)
                    nc.vector.memset(foi[:, (FR - 2) * FP // 2:
                                          (FR * FP + FTAIL) // 2], 0)
                    nc.vector.memset(fo[:, 2:FR - 2, 0:1], 0.0)
                    nc.vector.memset(fo[:, 2:FR - 2, FP - 3:FP], 0.0)
                psum1 = ps1.tile([128, NPX], F32, name="psum1", tag="psum1")
                k = 0
                for t in range(2):
                    for du in range(3):
                        for dv in range(3):
                            rhs = xpads[t][:,
                                           i * CROWS + du:
                                           i * CROWS + du + CROWS,
                                           dv + 1:dv + 1 + W]
                            blk = t * 9 + du * 3 + dv
                            lhsT = wext_sb[:, blk * 128:(blk + 1) * 128]
                            nc.tensor.matmul(psum1, lhsT, rhs,
                                             start=(k == 0), stop=(k == 17))
                            k += 1
                pv = psum1.rearrange("p (r c) -> p r c", c=W)
                dst_f = fr[:, i * CROWS + 2:i * CROWS + 2 + CROWS, 2:2 + W]
                nc.scalar.activation(dst_f, pv, AF.Identity,
                                     bias=bias_sb[:, 0:1], scale=1.0)
                dst_fo = fo[:, i * CROWS + 2:i * CROWS + 2 + CROWS, 1:1 + W]
                nc.scalar.activation(dst_fo, pv, AF.Identity,
                                     bias=bias_sb[:, 0:1], scale=1.0)
                if i % 2 == 1:
                    g = i // 2
                    base = (g * GROWS + 2) * FP
                    sq = prodp.tile([128, GROWS * FP], BF16,
                                    name=f"prod0_{g}", tag="prod0", bufs=4)
                    nc.scalar.activation(sq, fpad[:, base:base + GROWS * FP],
                                         AF.Square)
                    squares[g] = sq

            # ---- stage 2: products (2-chunk groups) + regressor GEMM ----
            # The two chunks of a group share each product map and run
            # concurrently on the PE's two column halves.
            for g in range(NGRP):
                ptiles = [squares[g]]
                for kk, (a, b) in enumerate(SYM):
                    if kk == 0:
                        continue
                    nrows = GROWS + a
                    base = (g * GROWS + 2 - a) * FP
                    pt = prodp.tile([128, nrows * FP], BF16,
                                    name=f"prod{kk}", tag=f"prod{kk}")
                    in0 = fpad[:, base:base + nrows * FP]
                    if b % 2 == 0:
                        in1 = fpad[:, base + a * FP + b:
                                   base + a * FP + b + nrows * FP]
                    else:
                        in1 = fodd[:, base + a * FP + b - 1:
                                   base + a * FP + b - 1 + nrows * FP]
                    nc.vector.tensor_mul(pt, in0, in1)
                    ptiles.append(pt)

                # Chunk A accumulates in its own bank (PSUM partitions
                # 0-63, column tile (0,0)); chunk B in a second bank
                # (partitions 64-127, column tile (0,64)) - separate banks
                # because a group-start's has_written clear is
                # bank-granular.
                cA, cB = 2 * g, 2 * g + 1
                psA = ps2a.tile([128, NPX], F32, name="psumA", tag="psumA")
                psB = ps2b.tile([128, NPX], F32, name="psumB", tag="psumB")
                mm = 0
                for kk, (a, b) in enumerate(SYM):
                    pr = ptiles[kk].rearrange("p (r c) -> p r c", c=FP)
                    taps = ([(a, b)] if (a, b) == (0, 0)
                            else [(a, b), (-a, -b)])
                    for (p, q) in taps:
                        if kk == 0:
                            rA = pr[:, 0:CROWS, 2:2 + W]
                            rB = pr[:, CROWS:2 * CROWS, 2:2 + W]
                        elif (p, q) == (a, b):
                            rA = pr[:, a:a + CROWS, 2:2 + W]
                            rB = pr[:, CROWS + a:CROWS + a + CROWS, 2:2 + W]
                        else:
                            rA = pr[:, 0:CROWS, 2 - b:2 - b + W]
                            rB = pr[:, CROWS:2 * CROWS, 2 - b:2 - b + W]
                        tidx = (p + 2) * 5 + (q + 2)
                        lhsT = wreg_sb[:, tidx * 64:(tidx + 1) * 64]
                        nc.tensor.matmul(psA[0:64], lhsT, rA,
                                         start=(mm == 0), stop=(mm == 24))
                        nc.tensor.matmul(psB[64:128], lhsT, rB,
                                         start=(mm == 0), stop=(mm == 24))
                        mm += 1

                # bias-copies; out DMAs split across the two HWDGE queues.
                # For the last group the drain is the kernel's critical
                # tail: run B's copies on VectorE concurrently with A's on
                # ScalarE, in half-column pieces, so the out DMAs (whose
                # completion semaphores gate the final barrier by ~1.5us)
                # issue as early as possible.
                outt = outp.tile([128, NPX], F32, name="outsb", tag="outsb")
                if g == NGRP - 1:
                    hp = NPX // 2
                    for h in range(2):
                        s = slice(h * hp, (h + 1) * hp)
                        nc.scalar.activation(outt[0:64, s], psA[0:64, s],
                                             AF.Identity,
                                             bias=bias_sb[0:64, 1:2],
                                             scale=1.0)
                        nc.vector.tensor_scalar_add(outt[64:128, s],
                                                    psB[64:128, s],
                                                    bias_sb[64:128, 1:2])
                        nc.sync.dma_start(
                            out=out[:, cA * NPX + h * hp:cA * NPX + h * hp
                                    + hp], in_=outt[0:64, s])
                        nc.scalar.dma_start(
                            out=out[:, cB * NPX + h * hp:cB * NPX + h * hp
                                    + hp], in_=outt[64:128, s])
                else:
                    nc.scalar.activation(outt[0:64], psA[0:64], AF.Identity,
                                         bias=bias_sb[0:64, 1:2], scale=1.0)
                    nc.scalar.activation(outt[64:128], psB[64:128],
                                         AF.Identity,
                                         bias=bias_sb[64:128, 1:2], scale=1.0)
                    nc.sync.dma_start(out=out[:, cA * NPX:(cA + 1) * NPX],
                                      in_=outt[0:64])
                    nc.scalar.dma_start(out=out[:, cB * NPX:(cB + 1) * NPX],
                                        in_=outt[64:128])


def build_nc():
    nc = bacc.Bacc("TRN2", target_bir_lowering=False, debug=False,
                   num_devices=NCORES)
    x = nc.dram_tensor("x", [CIN, XR * XP], BF16, kind="ExternalInput").ap()
    wext = nc.dram_tensor("wext", [128, 18 * 128], BF16,
                          kind="ExternalInput").ap()
    wreg = nc.dram_tensor("wreg", [128, 25 * 64], BF16,
                          kind="ExternalInput").ap()
    biases = nc.dram_tensor("biases", [128, 2], F32,
                            kind="ExternalInput").ap()
    out = nc.dram_tensor("out", [COUT, HW], F32, kind="ExternalOutput").ap()
    with tile.TileContext(nc) as tc:
        build_body(nc, tc, x, wext, wreg, biases, out)
    nc.compile()
    return nc


def prep_in_maps(x, w_ext, b_ext, w_reg, b_reg):
    x = np.ascontiguousarray(np.asarray(x, dtype=np.float32))
    w_ext = np.asarray(w_ext, dtype=np.float32)
    w_reg = np.asarray(w_reg, dtype=np.float32)
    b_ext = np.asarray(b_ext, dtype=np.float32)
    b_reg = np.asarray(b_reg, dtype=np.float32)

    # lhsT layouts: wext [cin(128-part), (cintile,tap)*cc], wreg [cc, tap*cout]
    w1 = np.transpose(w_ext, (1, 2, 3, 0))          # [CIN, 3, 3, CC]
    wext_p = np.zeros((128, 18, 128), np.float32)
    for t in range(2):
        for du in range(3):
            for dv in range(3):
                wext_p[:, t * 9 + du * 3 + dv, :] = \
                    w1[t * 128:(t + 1) * 128, du, dv, :]
    wext_p = np.ascontiguousarray(wext_p.reshape(128, 18 * 128).astype(BFNP))
    w2 = np.transpose(w_reg, (1, 2, 3, 0))          # [CC, 5, 5, COUT]
    wreg_p = np.ascontiguousarray(w2.reshape(128, 25 * 64).astype(BFNP))
    # biases packed in one tensor (one DMA): col 0 = b_ext; col 1 = b_reg
    # replicated across both column-tile halves (PSUM partitions 0-63 =
    # chunk A, 64-127 = chunk B)
    biases_p = np.ascontiguousarray(np.stack(
        [b_ext, np.concatenate([b_reg, b_reg])], axis=1))
    # x baked into the kernel's padded layout (top/bottom pad rows, data
    # at cols 2..65 so all SBUF row starts stay 4-byte aligned)
    xpad_h = np.zeros((B, CIN, XR, XP), BFNP)
    xpad_h[:, :, 1:1 + H, 2:2 + W] = x.astype(BFNP)
    xpad_h = xpad_h.reshape(B, CIN, XR * XP)

    return [{
        "x": np.ascontiguousarray(xpad_h[b]),
        "wext": wext_p,
        "wreg": wreg_p,
        "biases": biases_p,
    } for b in range(B)]


_NC_CACHE = None


def kernel(x, w_ext, b_ext, w_reg, b_reg):
    global _NC_CACHE
    if _NC_CACHE is None:
        _NC_CACHE = build_nc()
    nc = _NC_CACHE
    in_maps = prep_in_maps(x, w_ext, b_ext, w_reg, b_reg)
    res = run_bass_kernel_spmd(nc, in_maps, list(range(NCORES)))
    return np.stack([res.results[b]["out"].reshape(COUT, H, W)
                     for b in range(B)], axis=0)


# revision 26
# speedup vs baseline: 1.0365x; 1.0365x over previous
"""Trainium2 Bass kernel for nn_AutoCorr2D.

Computation (per sample):
  f   = conv3x3(x, w_ext, pad=1) + b_ext            # [CC=128, 64, 64]
  corr[c,i,j,k] = f[c,i,j] * fpad[c, i+u-2, j+v-2]  # 5x5 window products
  out[o,i,j]    = sum_{c,k} w_reg[o,c,k] * corr[c,i,j,k] + b_reg[o]

Sharding: data-parallel over batch B=8 across 8 NeuronCores (one sample per
core); conv weights replicated.

Per-core implementation (all matmul operands bf16, PSUM accumulation f32;
end-to-end rel err ~5e-3 vs the f32 reference, tolerance 2e-2):
  stage 1: implicit GEMM over (cin_tile, 3x3 tap): 18 accumulating bf16
           matmuls per 512-pixel chunk reading shifted views of a
           zero-padded x buffer (DMA'd as bf16, pad-scattered by VectorE
           at 4x rate); bias folded into two PSUM->SBUF ScalarE copies
           that write the feature map twice: fpad (col offset 2) and fodd
           (col offset 1) so every stage-2 product has 4B-aligned operands
           (DVE 2x_1P mode needs 16-bit dtype + 4B alignment; taps with
           odd column shift read their shifted operand from fodd).
  stage 2: product symmetry: P_{a,b}[y,x] = f[y,x]*f[y+a,x+b] serves both
           tap (a,b) and tap (-a,-b) via shifted reads, so only 13 of 25
           product maps are computed per 2-chunk row group (ScalarE Square
           for (0,0) - emitted inside the stage-1 loop to dodge FIFO
           head-of-line blocking - VectorE tensor_mul at 2x for the rest).
           The regressor GEMM has M=COUT=64 - half the PE array - so the
           two chunks of each group run CONCURRENTLY via 2x column tiling:
           chunk A accumulates 25 taps into PSUM partitions 0-63
           (tile_position (0,0)), chunk B into partitions 64-127
           (tile_position (0,64)) of the same bank; one 128-partition
           bias-copy then two DMAs write both chunks out.
  The PE is pre-warmed with dummy matmuls so the HAM clock gate releases
  before real work.
"""

import numpy as np
import ml_dtypes

from concourse import bacc, mybir, tile
from concourse.bass_utils import run_bass_kernel_spmd

B, CIN, H, W = 8, 256, 64, 64
CC, COUT = 128, 64
HW = H * W
NCORES = 8

NCHUNK = 8           # pixel chunks per image
CROWS = H // NCHUNK  # rows per chunk (8) -> N = 512 pixels
NPX = CROWS * W      # 512
NGRP = 4             # product-map groups (2 chunks each)
GROWS = 2 * CROWS    # 16

XP = W + 4           # xpad cols: data at 2..65, zeros at 0,1,66,67 (4B align)
XR = H + 2           # xpad rows (pad=1)
FP = W + 4           # fpad cols (pad=2)
FR = H + 4           # fpad rows
FTAIL = 72           # guard tail so shifted product reads stay in-bounds

# The 13 "upper half" taps; (a,b) also serves tap (-a,-b) via a shifted read.
SYM = [(0, 0), (0, 1), (0, 2),
       (1, -2), (1, -1), (1, 0), (1, 1), (1, 2),
       (2, -2), (2, -1), (2, 0), (2, 1), (2, 2)]

F32 = mybir.dt.float32
BF16 = mybir.dt.bfloat16
U32 = mybir.dt.uint32
AF = mybir.ActivationFunctionType
BFNP = ml_dtypes.bfloat16


def build_body(nc, tc, x, wext, wreg, biases, out):
    with (
        tc.tile_pool(name="const", bufs=1) as constp,
        tc.tile_pool(name="xpadp", bufs=1) as xpadp,
        tc.tile_pool(name="fpadp", bufs=1) as fpadp,
        tc.tile_pool(name="prodp", bufs=3) as prodp,
        tc.tile_pool(name="outp", bufs=2) as outp,
        tc.tile_pool(name="ps1", bufs=3, space="PSUM") as ps1,
        tc.tile_pool(name="ps2a", bufs=2, space="PSUM") as ps2a,
        tc.tile_pool(name="ps2b", bufs=2, space="PSUM") as ps2b,
        tc.tile_pool(name="warmp", bufs=1, space="PSUM") as warmp,
    ):
        # PE warm-up: dummy matmuls on a zeroed bf16 scratch start immediately
        # and release the HAM clock gate (~3.4us of activity) before real
        # matmuls begin.
        wsc = constp.tile([128, NPX], BF16, name="wsc")
        nc.vector.memset(wsc.bitcast(U32), 0)
        wpsum = warmp.tile([128, NPX], F32, name="wpsum")
        NWARM = 7   # ends right when band 0 is scattered and wext block 0 up
        for i in range(NWARM):
            nc.tensor.matmul(wpsum, wsc[:, :128], wsc,
                             start=(i == 0), stop=(i == NWARM - 1))

        # ---- x pad buffers; borders zeroed once (u32-bitcast halves the
        # element count).  Data cols 2..65 keep the DVE pad-scatter 4B
        # aligned (4x tensor_copy mode).
        xpads = []
        for t in range(2):
            xp = xpadp.tile([128, XR * XP], BF16, name=f"xpad{t}",
                            tag=f"xpad{t}")
            xr = xp.rearrange("p (r c) -> p r c", c=XP)
            xri = xp.bitcast(U32).rearrange("p (r c) -> p r c", c=XP // 2)
            nc.vector.memset(xri[:, 0, :], 0)
            nc.vector.memset(xri[:, XR - 1, :], 0)
            nc.vector.memset(xri[:, 1:XR - 1, 0], 0)
            nc.vector.memset(xri[:, 1:XR - 1, XP // 2 - 1], 0)
            xpads.append(xr)

        # DMA queue plan (2 HWDGE queues: Sync + Scalar, ~90-120 GB/s
        # each): Sync carries only the x bands, in chunk order, so chunk 0
        # starts as early as possible; Scalar carries the weights in PE
        # consumption order, then the bias tensor and wreg, then stage
        # 2's B-half output DMAs.  Bands go through a staging tile + DVE
        # pad-scatter rather than gating matmuls on the DMA completion
        # semaphore directly (~1us DGE latency; the DVE copy's completion
        # broadcasts fast).
        wext_sb = constp.tile([128, 18 * 128], BF16, name="wext_sb")
        wreg_sb = constp.tile([128, 25 * 64], BF16, name="wreg_sb")
        bias_sb = constp.tile([128, 2], F32, name="bias_sb")
        for lo, hi in ((0, 3), (3, 9), (9, 13), (13, 18)):
            nc.scalar.dma_start(out=wext_sb[:, lo * 128:hi * 128],
                                in_=wext[:, lo * 128:hi * 128])

        # x bands are 1:1 with stage-1 chunks: band i carries exactly the
        # input rows chunk i reads (i*8-1 .. i*8+9, overlapping by 2), so
        # each chunk waits on one small just-in-time DMA + pad-scatter.
        with tc.tile_pool(name="xstagep", bufs=3) as xstagep:
            xsts = []
            band_rows = []
            for band in range(NCHUNK):
                ra = max(band * CROWS - 1, 0)
                rb = min(band * CROWS + CROWS + 1, H)
                band_rows.append((ra, rb))
                pair = []
                for t in range(2):
                    xst = xstagep.tile([128, (rb - ra) * W], BF16,
                                       name=f"xst{band}_{t}", tag="xst",
                                       padded_shape=[128, 10 * W])
                    src = x[t * 128:(t + 1) * 128, ra * W:rb * W]
                    nc.sync.dma_start(out=xst, in_=src)
                    pair.append(xst)
                xsts.append(pair)
                if band == 0:
                    nc.scalar.dma_start(out=bias_sb, in_=biases)
                    nc.scalar.dma_start(out=wreg_sb, in_=wreg)

            def scatter_band(band):
                # pad-scatter on VectorE (4x bf16 tensor_copy): keeps
                # ScalarE's FIFO free for the per-chunk bias-copies
                ra, rb = band_rows[band]
                for t in range(2):
                    dst = xpads[t][:, 1 + ra:1 + rb, 2:2 + W]
                    stv = xsts[band][t].rearrange("p (r c) -> p r c", c=W)
                    nc.vector.tensor_copy(dst, stv)
            # ---- padded features (pad=2) + guard tail; fodd is the same
            # map shifted one column left (fodd[r,x] = fpad[r,x+1]) so
            # odd-column-shift product operands stay 4B-aligned ----
            fpad = fpadp.tile([128, FR * FP + FTAIL], BF16, name="fpad")
            fodd = fpadp.tile([128, FR * FP + FTAIL], BF16, name="fodd")
            fr = fpad[:, :FR * FP].rearrange("p (r c) -> p r c", c=FP)
            fo = fodd[:, :FR * FP].rearrange("p (r c) -> p r c", c=FP)

            # ---- stage 1: f = conv3x3(x) + b_ext ----
            # fpad/fodd border memsets go first in VectorE's FIFO (cheap,
            # products need them later); the center-tap Square for group g
            # is emitted right after chunk 2g+1's bias-copies so ScalarE
            # reaches it long before the PE needs it (no head-of-line
            # blocking behind later chunks' copies).
            squares = [None] * NGRP
            for i in range(NCHUNK):
                scatter_band(i)
                if i == 0:
                    fpi = fpad.bitcast(U32)
                    foi = fodd.bitcast(U32)
                    fri = fpi[:, :FR * FP // 2].rearrange(
                        "p (r c) -> p r c", c=FP // 2)
                    nc.vector.memset(fpi[:, 0:FP], 0)
                    nc.vector.memset(fpi[:, (FR - 2) * FP // 2:
                                          (FR * FP + FTAIL) // 2], 0)
                    nc.vector.memset(fri[:, 2:FR - 2, 0], 0)
                    nc.vector.memset(fri[:, 2:FR - 2, FP // 2 - 1], 0)
                    nc.vector.memset(foi[:, 0:FP], 0)
                    nc.vector.memset(foi[:, (FR - 2) * FP // 2:
                                          (FR * FP + FTAIL) // 2], 0)
                    nc.vector.memset(fo[:, 2:FR - 2, 0:1], 0.0)
                    nc.vector.memset(fo[:, 2:FR - 2, FP - 3:FP], 0.0)
                psum1 = ps1.tile([128, NPX], F32, name="psum1", tag="psum1")
                k = 0
                for t in range(2):
                    for du in range(3):
                        for dv in range(3):
                            rhs = xpads[t][:,
                                           i * CROWS + du:
                                           i * CROWS + du + CROWS,
                                           dv + 1:dv + 1 + W]
                            blk = t * 9 + du * 3 + dv
                            lhsT = wext_sb[:, blk * 128:(blk + 1) * 128]
                            nc.tensor.matmul(psum1, lhsT, rhs,
                                             start=(k == 0), stop=(k == 17))
                            k += 1
                pv = psum1.rearrange("p (r c) -> p r c", c=W)
                dst_f = fr[:, i * CROWS + 2:i * CROWS + 2 + CROWS, 2:2 + W]
                nc.scalar.activation(dst_f, pv, AF.Identity,
                                     bias=bias_sb[:, 0:1], scale=1.0)
                dst_fo = fo[:, i * CROWS + 2:i * CROWS + 2 + CROWS, 1:1 + W]
                nc.scalar.activation(dst_fo, pv, AF.Identity,
                                     bias=bias_sb[:, 0:1], scale=1.0)
                if i % 2 == 1:
                    g = i // 2
                    base = (g * GROWS + 2) * FP
                    sq = prodp.tile([128, GROWS * FP], BF16,
                                    name=f"prod0_{g}", tag="prod0", bufs=4)
                    nc.scalar.activation(sq, fpad[:, base:base + GROWS * FP],
                                         AF.Square)
                    squares[g] = sq

            # ---- stage 2: products (2-chunk groups) + regressor GEMM ----
            # The two chunks of a group share each product map and run
            # concurrently on the PE's two column halves.
            for g in range(NGRP):
                ptiles = [squares[g]]
                for kk, (a, b) in enumerate(SYM):
                    if kk == 0:
                        continue
                    nrows = GROWS + a
                    base = (g * GROWS + 2 - a) * FP
                    pt = prodp.tile([128, nrows * FP], BF16,
                                    name=f"prod{kk}", tag=f"prod{kk}")
                    in0 = fpad[:, base:base + nrows * FP]
                    if b % 2 == 0:
                        in1 = fpad[:, base + a * FP + b:
                                   base + a * FP + b + nrows * FP]
                    else:
                        in1 = fodd[:, base + a * FP + b - 1:
                                   base + a * FP + b - 1 + nrows * FP]
                    nc.vector.tensor_mul(pt, in0, in1)
                    ptiles.append(pt)

                # Chunk A accumulates in its own bank (PSUM partitions
                # 0-63, column tile (0,0)); chunk B in a second bank
                # (partitions 64-127, column tile (0,64)) - separate banks
                # because a group-start's has_written clear is
                # bank-granular.
                cA, cB = 2 * g, 2 * g + 1
                psA = ps2a.tile([128, NPX], F32, name="psumA", tag="psumA")
                psB = ps2b.tile([128, NPX], F32, name="psumB", tag="psumB")
                mm = 0
                for kk, (a, b) in enumerate(SYM):
                    pr = ptiles[kk].rearrange("p (r c) -> p r c", c=FP)
                    taps = ([(a, b)] if (a, b) == (0, 0)
                            else [(a, b), (-a, -b)])
                    for (p, q) in taps:
                        if kk == 0:
                            rA = pr[:, 0:CROWS, 2:2 + W]
                            rB = pr[:, CROWS:2 * CROWS, 2:2 + W]
                        elif (p, q) == (a, b):
                            rA = pr[:, a:a + CROWS, 2:2 + W]
                            rB = pr[:, CROWS + a:CROWS + a + CROWS, 2:2 + W]
                        else:
                            rA = pr[:, 0:CROWS, 2 - b:2 - b + W]
                            rB = pr[:, CROWS:2 * CROWS, 2 - b:2 - b + W]
                        tidx = (p + 2) * 5 + (q + 2)
                        lhsT = wreg_sb[:, tidx * 64:(tidx + 1) * 64]
                        nc.tensor.matmul(psA[0:64], lhsT, rA,
                                         start=(mm == 0), stop=(mm == 24))
                        nc.tensor.matmul(psB[64:128], lhsT, rB,
                                         start=(mm == 0), stop=(mm == 24))
                        mm += 1

                # bias-copies; out DMAs split across the two HWDGE queues.
                # For the last group the drain is the kernel's critical
                # tail: run B's copies on VectorE concurrently with A's on
                # ScalarE, in half-column pieces, so the out DMAs (whose
                # completion semaphores gate the final barrier by ~1.5us)
                # issue as early as possible.
                outt = outp.tile([128, NPX], F32, name="outsb", tag="outsb")
                if g == NGRP - 1:
                    hp = NPX // 2
                    for h in range(2):
                        s = slice(h * hp, (h + 1) * hp)
                        nc.scalar.activation(outt[0:64, s], psA[0:64, s],
                                             AF.Identity,
                                             bias=bias_sb[0:64, 1:2],
                                             scale=1.0)
                        nc.vector.tensor_scalar_add(outt[64:128, s],
                                                    psB[64:128, s],
                                                    bias_sb[64:128, 1:2])
                        nc.sync.dma_start(
                            out=out[:, cA * NPX + h * hp:cA * NPX + h * hp
                                    + hp], in_=outt[0:64, s])
                        nc.scalar.dma_start(
                            out=out[:, cB * NPX + h * hp:cB * NPX + h * hp
                                    + hp], in_=outt[64:128, s])
                else:
                    nc.scalar.activation(outt[0:64], psA[0:64], AF.Identity,
                                         bias=bias_sb[0:64, 1:2], scale=1.0)
                    nc.scalar.activation(outt[64:128], psB[64:128],
                                         AF.Identity,
                                         bias=bias_sb[64:128, 1:2], scale=1.0)
                    nc.sync.dma_start(out=out[:, cA * NPX:(cA + 1) * NPX],
                                      in_=outt[0:64])
                    nc.scalar.dma_start(out=out[:, cB * NPX:(cB + 1) * NPX],
                                        in_=outt[64:128])


def build_nc():
    nc = bacc.Bacc("TRN2", target_bir_lowering=False, debug=False,
                   num_devices=NCORES)
    x = nc.dram_tensor("x", [CIN, HW], BF16, kind="ExternalInput").ap()
    wext = nc.dram_tensor("wext", [128, 18 * 128], BF16,
                          kind="ExternalInput").ap()
    wreg = nc.dram_tensor("wreg", [128, 25 * 64], BF16,
                          kind="ExternalInput").ap()
    biases = nc.dram_tensor("biases", [128, 2], F32,
                            kind="ExternalInput").ap()
    out = nc.dram_tensor("out", [COUT, HW], F32, kind="ExternalOutput").ap()
    with tile.TileContext(nc) as tc:
        build_body(nc, tc, x, wext, wreg, biases, out)
    nc.compile()
    return nc


def prep_in_maps(x, w_ext, b_ext, w_reg, b_reg):
    x = np.ascontiguousarray(np.asarray(x, dtype=np.float32))
    w_ext = np.asarray(w_ext, dtype=np.float32)
    w_reg = np.asarray(w_reg, dtype=np.float32)
    b_ext = np.asarray(b_ext, dtype=np.float32)
    b_reg = np.asarray(b_reg, dtype=np.float32)

    # lhsT layouts: wext [cin(128-part), (cintile,tap)*cc], wreg [cc, tap*cout]
    w1 = np.transpose(w_ext, (1, 2, 3, 0))          # [CIN, 3, 3, CC]
    wext_p = np.zeros((128, 18, 128), np.float32)
    for t in range(2):
        for du in range(3):
            for dv in range(3):
                wext_p[:, t * 9 + du * 3 + dv, :] = \
                    w1[t * 128:(t + 1) * 128, du, dv, :]
    wext_p = np.ascontiguousarray(wext_p.reshape(128, 18 * 128).astype(BFNP))
    w2 = np.transpose(w_reg, (1, 2, 3, 0))          # [CC, 5, 5, COUT]
    wreg_p = np.ascontiguousarray(w2.reshape(128, 25 * 64).astype(BFNP))
    # biases packed in one tensor (one DMA): col 0 = b_ext; col 1 = b_reg
    # replicated across both column-tile halves (PSUM partitions 0-63 =
    # chunk A, 64-127 = chunk B)
    biases_p = np.ascontiguousarray(np.stack(
        [b_ext, np.concatenate([b_reg, b_reg])], axis=1))
    xb = x.astype(BFNP)

    return [{
        "x": np.ascontiguousarray(xb[b].reshape(CIN, HW)),
        "wext": wext_p,
        "wreg": wreg_p,
        "biases": biases_p,
    } for b in range(B)]


_NC_CACHE = None


def kernel(x, w_ext, b_ext, w_reg, b_reg):
    global _NC_CACHE
    if _NC_CACHE is None:
        _NC_CACHE = build_nc()
    nc = _NC_CACHE
    in_maps = prep_in_maps(x, w_ext, b_ext, w_reg, b_reg)
    res = run_bass_kernel_spmd(nc, in_maps, list(range(NCORES)))
    return np.stack([res.results[b]["out"].reshape(COUT, H, W)
                     for b in range(B)], axis=0)
